# revision 1
# baseline (speedup 1.0000x reference)
"""EMA head kernel for Trainium2 (Bass/Tile), 8 NeuronCores.

Problem: alpha = clip(sigmoid(MLP(feat)), 0.01, 0.99) per (t, b);
         y[0] = r[0]; y[t] = (1-alpha[t])*y[t-1] + alpha[t]*r[t].

Sharding: time dim T=4096 split into 8 slabs of 512 (all B=256 per core).
Each core computes, for its slab, the local affine-scan pieces
    z[t] = A[t]*z[t-1] + Bv[t]   (z[-1] = 0),   A = 1-alpha, Bv = alpha*r
    P[t] = A[t]*P[t-1]           (P[-1] = 1)
and the host stitches slabs with   y = z + P * carry,  carry' = y[-1].
carry_0 = r[0] reproduces y[0] = r[0] exactly: a*r + (1-a)*r = r.

On-chip layout: feat tiles [128 b, 128 f] (contiguous DMA), PE transpose ->
featT [f, b] (PSUM), copy to SBUF, matmul lhsT=featT rhs=W1 -> h [b, 16]
collected 32 t-steps per PSUM bank, then +b1/relu/*W2/reduce on ACT+DVE in
[128, 512] batches -> alpha_pre [128 b, t], sigmoid+clip, and
tensor_tensor_scan along the free (t) dim for z and P.
"""

import numpy as np

T, B, FEAT, HID = 4096, 256, 128, 16
NCORES = 8
TLOC = T // NCORES  # 512
NH = 2              # batch halves of 128
TG = 8              # t-steps per feat dma_start (1 MB)
# engine assignment for the fp32->fp16 cast of each feat group
CAST_PATTERN = ["g", "g", "v", "g", "g", "s", "g", "v"]

_CACHE = {}


def _build_program():
    import concourse.bacc as bacc
    import concourse.bass as bass
    import concourse.tile as tile
    from concourse import mybir
    from concourse.masks import make_identity

    fp32 = mybir.dt.float32
    fp16 = mybir.dt.float16
    AF = mybir.ActivationFunctionType
    OP = mybir.AluOpType

    nc = bacc.Bacc("TRN2", target_bir_lowering=False, debug=False,
                   num_devices=NCORES)

    feat_d = nc.dram_tensor("feat", [TLOC, B, FEAT], fp32, kind="ExternalInput")
    r_d = nc.dram_tensor("r", [TLOC, B], fp32, kind="ExternalInput")
    w1_d = nc.dram_tensor("w1", [FEAT, HID], fp32, kind="ExternalInput")
    b1_d = nc.dram_tensor("b1", [HID], fp32, kind="ExternalInput")
    w2_d = nc.dram_tensor("w2", [HID], fp32, kind="ExternalInput")
    b2_d = nc.dram_tensor("b2", [1], fp32, kind="ExternalInput")
    z_d = nc.dram_tensor("z", [NH, 128, TLOC], fp32, kind="ExternalOutput")
    p_d = nc.dram_tensor("p", [NH, 128, TLOC], fp32, kind="ExternalOutput")

    with tile.TileContext(nc) as tc:
        with (
            tc.tile_pool(name="singles", bufs=1) as singles,
            tc.tile_pool(name="featin", bufs=3) as featin,
            tc.tile_pool(name="ftps", bufs=3, space="PSUM") as ftps,
            tc.tile_pool(name="hps", bufs=2, space="PSUM") as hps,
            tc.tile_pool(name="ftsb", bufs=3) as ftsb,
            tc.tile_pool(name="hwork", bufs=2) as hwork,
        ):
            # ---------------- constants ----------------
            ident = singles.tile([128, 128], fp16)
            make_identity(nc, ident)
            ident32 = singles.tile([128, 128], fp32)
            make_identity(nc, ident32)
            w1_sb = singles.tile([128, HID], fp16)
            nc.gpsimd.dma_start(w1_sb, w1_d[:, :])
            b1rep = singles.tile([128, 32, HID], fp32)
            nc.gpsimd.dma_start(
                b1rep, bass.AP(b1_d, 0, [[0, 128], [0, 32], [1, HID]]))
            w2rep = singles.tile([128, 32, HID], fp32)
            nc.gpsimd.dma_start(
                w2rep, bass.AP(w2_d, 0, [[0, 128], [0, 32], [1, HID]]))
            b2col = singles.tile([128, 1], fp32)
            nc.gpsimd.dma_start(b2col, bass.AP(b2_d, 0, [[0, 128], [1, 1]]))
            ones_sb = singles.tile([128, TLOC], fp32)
            nc.vector.memset(ones_sb, 1.0)

            # ---- r: load [t, b]; PE-transpose to rT [b, t] per half ----
            rT = [singles.tile([128, TLOC], fp32, tag=f"rT{h}", name=f"rT{h}")
                  for h in range(NH)]
            for tcnk in range(TLOC // 128):
                rload = featin.tile([128, B], fp32, tag="rload")
                nc.sync.dma_start(rload, r_d[tcnk * 128:(tcnk + 1) * 128, :])
                rps = ftps.tile([128, 4, 128], fp32, tag="ftp16")
                rview = rload[:, :].rearrange("p (b j) -> p j b", j=2)
                for h in range(NH):
                    nc.tensor.transpose(rps[:, h, :], rview[:, h, :], ident32)
                for h in range(NH):
                    nc.vector.tensor_copy(
                        rT[h][:, tcnk * 128:(tcnk + 1) * 128], rps[:, h, :])

            # per-half alpha_pre accumulators [128 b, t]
            apre = [singles.tile([128, TLOC], fp32, tag=f"apre{h}", name=f"apre{h}")
                    for h in range(NH)]

            # ---------------- main feat pipeline ----------------
            hbank = [None, None]
            copy_parity = 0
            for t0 in range(0, TLOC, TG):
                if t0 % 32 == 0:
                    hbank[0] = hps.tile([128, 32, HID], fp32, tag="h0", name="hbank0")
                    hbank[1] = hps.tile([128, 32, HID], fp32, tag="h1", name="hbank1")

                fin = featin.tile([128, TG, 2 * FEAT], fp16, tag="fin")
                nc.gpsimd.dma_start(
                    fin,
                    feat_d[t0:t0 + TG, :, :].rearrange(
                        "t (p j) f -> p t (j f)", j=2))

                # groups of 4 tiles: transpose -> psum bank -> copy -> matmul
                for q in range(0, 2 * TG, 4):
                    ftp = ftps.tile([128, 4, 128], fp16, tag="ftp16")
                    for s in range(4):
                        g = q + s
                        tt, j = g // 2, g % 2
                        nc.tensor.transpose(
                            ftp[:, s, :],
                            fin[:, tt, j * FEAT:(j + 1) * FEAT], ident)
                    fts = ftsb.tile([128, 4, 128], fp16, tag="fts")
                    if copy_parity == 0:
                        nc.vector.tensor_copy(fts, ftp)
                    else:
                        nc.scalar.copy(fts, ftp)
                    copy_parity ^= 1
                    for s in range(4):
                        g = q + s
                        tt, h = g // 2, g % 2
                        slot = (t0 + tt) % 32
                        nc.tensor.matmul(hbank[h][:, slot, :],
                                         fts[:, s, :], w1_sb)

                if (t0 + TG) % 32 == 0:
                    blk = t0 // 32
                    for h in range(NH):
                        hb = hwork.tile([128, 32, HID], fp32, tag="hb")
                        nc.vector.tensor_add(hb, hbank[h], b1rep)
                        hrelu = hwork.tile([128, 32, HID], fp32, tag="hrelu")
                        nc.scalar.activation(hrelu, hb, AF.Relu)
                        hw = hwork.tile([128, 32, HID], fp32, tag="hw")
                        nc.vector.tensor_mul(hw, hrelu, w2rep)
                        nc.vector.tensor_reduce(
                            apre[h][:, blk * 32:(blk + 1) * 32],
                            hw, axis=mybir.AxisListType.X, op=OP.add)

            # ---------------- alpha -> scans -> out ----------------
            for h in range(NH):
                alpha = singles.tile([128, TLOC], fp32, tag=f"alpha{h}")
                nc.scalar.activation(alpha, apre[h], AF.Sigmoid, bias=b2col)
                nc.vector.tensor_scalar(alpha, alpha, 0.01, 0.99,
                                        op0=OP.max, op1=OP.min)
                A_sb = singles.tile([128, TLOC], fp32, tag=f"A{h}")
                nc.vector.tensor_scalar(A_sb, alpha, -1.0, 1.0,
                                        op0=OP.mult, op1=OP.add)
                Bv = singles.tile([128, TLOC], fp32, tag=f"Bv{h}")
                nc.vector.tensor_mul(Bv, alpha, rT[h])
                z_sb = singles.tile([128, TLOC], fp32, tag=f"z{h}")
                nc.vector.tensor_tensor_scan(z_sb, A_sb, Bv, 0.0,
                                             op0=OP.mult, op1=OP.add)
                p_sb = singles.tile([128, TLOC], fp32, tag=f"p{h}")
                nc.vector.tensor_tensor_scan(p_sb, A_sb, ones_sb, 1.0,
                                             op0=OP.mult, op1=OP.mult)
                nc.sync.dma_start(z_d[h], z_sb)
                nc.sync.dma_start(p_d[h], p_sb)

    nc.finalize()
    return nc


def _get_program():
    if "nc" not in _CACHE:
        _CACHE["nc"] = _build_program()
    return _CACHE["nc"]


def kernel(r, feat, W1, b1, W2, b2, _run_kwargs=None, _return_results=False):
    from concourse.bass_utils import run_bass_kernel_spmd

    r = np.asarray(r, dtype=np.float32)
    feat = np.asarray(feat, dtype=np.float32)
    W1 = np.asarray(W1, dtype=np.float32)
    b1 = np.asarray(b1, dtype=np.float32).reshape(HID)
    W2 = np.asarray(W2, dtype=np.float32).reshape(HID)
    b2 = np.asarray(b2, dtype=np.float32).reshape(1)

    nc = _get_program()
    in_maps = []
    for c in range(NCORES):
        in_maps.append({
            "feat": np.ascontiguousarray(feat[c * TLOC:(c + 1) * TLOC]),
            "r": np.ascontiguousarray(r[c * TLOC:(c + 1) * TLOC, :, 0]),
            "w1": W1, "b1": b1, "w2": W2, "b2": b2,
        })

    kw = _run_kwargs or {}
    res = run_bass_kernel_spmd(nc, in_maps, core_ids=list(range(NCORES)), **kw)

    # host stitch: y = z + P*carry per slab, carry chain across slabs
    y = np.empty((T, B), dtype=np.float32)
    carry = r[0, :, 0].astype(np.float32)
    for c in range(NCORES):
        zc = res.results[c]["z"].transpose(2, 1, 0).reshape(TLOC, B)
        pc = res.results[c]["p"].transpose(2, 1, 0).reshape(TLOC, B)
        y_slab = zc + pc * carry[None, :]
        carry = y_slab[-1]
        y[c * TLOC:(c + 1) * TLOC] = y_slab
    out = y[:, :, None]
    if _return_results:
        return out, res
    return out



# revision 3
# speedup vs baseline: 1.5792x; 1.5792x over previous
"""EMA head kernel for Trainium2 (Bass/Tile), 8 NeuronCores.

Problem: alpha = clip(sigmoid(MLP(feat)), 0.01, 0.99) per (t, b);
         y[0] = r[0]; y[t] = (1-alpha[t])*y[t-1] + alpha[t]*r[t].

Sharding: time dim T=4096 split into 8 slabs of 512 (all B=256 per core).
Each core computes, for its slab, the local affine-scan pieces
    z[t] = A[t]*z[t-1] + Bv[t]   (z[-1] = 0),   A = 1-alpha, Bv = alpha*r
    P[t] = A[t]*P[t-1]           (P[-1] = 1)
and the host stitches slabs with   y = z + P * carry,  carry' = y[-1].
carry_0 = r[0] reproduces y[0] = r[0] exactly: a*r + (1-a)*r = r.

v2: feat is pre-transposed AND down-cast on the host to
    featT [2 (b-half j), 128 (f), TLOC (t), 128 (b)]  (fp16 or fp8).
Device DMA is then fully contiguous (16 KB per partition per chunk) and
the PE transpose pass + PSUM->SBUF copies of v1 disappear entirely:
matmul lhsT = featT tile [f, b] directly, rhs = W1 [f, 16]
-> h [128 b, 16] per t in PSUM, then +b1/relu/*W2/reduce -> alpha_pre,
sigmoid+clip, tensor_tensor_scan along t for z and P (as v1).
r is likewise pre-transposed on the host to rT [2, 128 b, TLOC] fp32.
"""

import numpy as np

T, B, FEAT, HID = 4096, 256, 128, 16
NCORES = 8
TLOC = T // NCORES  # 512
NH = 2              # batch halves of 128
CH = 64             # t-steps per feat DMA chunk (2 MB @ fp16)
SB = 32             # t-steps per PSUM h-bank (one 2KB bank)

FEAT_FP8 = False    # False: fp16 feat/W1.  True: fp8 (e4m3) feat/W1.

_CACHE = {}


def _np_feat_dtype():
    if FEAT_FP8:
        import ml_dtypes
        return ml_dtypes.float8_e4m3
    return np.float16


def _build_program():
    import concourse.bacc as bacc
    import concourse.bass as bass
    import concourse.tile as tile
    from concourse import mybir

    fp32 = mybir.dt.float32
    fdt = mybir.dt.float8e4 if FEAT_FP8 else mybir.dt.float16
    AF = mybir.ActivationFunctionType
    OP = mybir.AluOpType

    nc = bacc.Bacc("TRN2", target_bir_lowering=False, debug=False,
                   num_devices=NCORES)

    featT_d = nc.dram_tensor("featT", [NH, FEAT, TLOC, 128], fdt,
                             kind="ExternalInput")
    rT_d = nc.dram_tensor("rT", [NH, 128, TLOC], fp32, kind="ExternalInput")
    w1_d = nc.dram_tensor("w1", [FEAT, HID], fdt, kind="ExternalInput")
    b1_d = nc.dram_tensor("b1", [HID], fp32, kind="ExternalInput")
    w2_d = nc.dram_tensor("w2", [HID], fp32, kind="ExternalInput")
    b2_d = nc.dram_tensor("b2", [1], fp32, kind="ExternalInput")
    z_d = nc.dram_tensor("z", [NH, 128, TLOC], fp32, kind="ExternalOutput")
    p_d = nc.dram_tensor("p", [NH, 128, TLOC], fp32, kind="ExternalOutput")

    with tile.TileContext(nc) as tc:
        with (
            tc.tile_pool(name="singles", bufs=1) as singles,
            tc.tile_pool(name="featin", bufs=3) as featin,
            tc.tile_pool(name="hps", bufs=4, space="PSUM") as hps,
            tc.tile_pool(name="hwork", bufs=3) as hwork,
        ):
            # ---------------- constants ----------------
            w1_sb = singles.tile([128, HID], fdt)
            nc.sync.dma_start(w1_sb, w1_d[:, :])
            b1rep = singles.tile([128, SB, HID], fp32)
            nc.gpsimd.dma_start(
                b1rep, bass.AP(b1_d, 0, [[0, 128], [0, SB], [1, HID]]))
            w2rep = singles.tile([128, SB, HID], fp32)
            nc.gpsimd.dma_start(
                w2rep, bass.AP(w2_d, 0, [[0, 128], [0, SB], [1, HID]]))
            b2col = singles.tile([128, 1], fp32)
            nc.gpsimd.dma_start(b2col, bass.AP(b2_d, 0, [[0, 128], [1, 1]]))
            ones_sb = singles.tile([128, TLOC], fp32)
            nc.vector.memset(ones_sb, 1.0)

            rT = [singles.tile([128, TLOC], fp32, tag=f"rT{h}", name=f"rT{h}")
                  for h in range(NH)]
            for h in range(NH):
                nc.sync.dma_start(rT[h], rT_d[h])

            apre = [singles.tile([128, TLOC], fp32, tag=f"apre{h}",
                                 name=f"apre{h}")
                    for h in range(NH)]

            # ---------------- main feat pipeline ----------------
            dma_parity = 0
            for j in range(NH):
                for t0 in range(0, TLOC, CH):
                    ft = featin.tile([128, CH, 128], fdt, tag="ft")
                    eng = nc.sync if dma_parity == 0 else nc.gpsimd
                    dma_parity ^= 1
                    eng.dma_start(ft, featT_d[j, :, t0:t0 + CH, :])
                    for s2 in range(CH // SB):
                        hbank = hps.tile([128, SB, HID], fp32, tag="hb")
                        for s in range(SB):
                            nc.tensor.matmul(hbank[:, s, :],
                                             ft[:, s2 * SB + s, :], w1_sb)
                        blk = t0 + s2 * SB
                        hb = hwork.tile([128, SB, HID], fp32, tag="hadd")
                        nc.vector.tensor_add(hb, hbank, b1rep)
                        hrelu = hwork.tile([128, SB, HID], fp32, tag="hrelu")
                        nc.scalar.activation(hrelu, hb, AF.Relu)
                        hw = hwork.tile([128, SB, HID], fp32, tag="hw")
                        nc.vector.tensor_mul(hw, hrelu, w2rep)
                        nc.vector.tensor_reduce(
                            apre[j][:, blk:blk + SB], hw,
                            axis=mybir.AxisListType.X, op=OP.add)

                # ---------------- alpha -> scans -> out (half j) --------
                alpha = singles.tile([128, TLOC], fp32, tag=f"alpha{j}")
                nc.scalar.activation(alpha, apre[j], AF.Sigmoid, bias=b2col)
                nc.vector.tensor_scalar(alpha, alpha, 0.01, 0.99,
                                        op0=OP.max, op1=OP.min)
                A_sb = singles.tile([128, TLOC], fp32, tag=f"A{j}")
                nc.vector.tensor_scalar(A_sb, alpha, -1.0, 1.0,
                                        op0=OP.mult, op1=OP.add)
                Bv = singles.tile([128, TLOC], fp32, tag=f"Bv{j}")
                nc.vector.tensor_mul(Bv, alpha, rT[j])
                z_sb = singles.tile([128, TLOC], fp32, tag=f"z{j}")
                nc.vector.tensor_tensor_scan(z_sb, A_sb, Bv, 0.0,
                                             op0=OP.mult, op1=OP.add)
                p_sb = singles.tile([128, TLOC], fp32, tag=f"p{j}")
                nc.vector.tensor_tensor_scan(p_sb, A_sb, ones_sb, 1.0,
                                             op0=OP.mult, op1=OP.mult)
                nc.sync.dma_start(z_d[j], z_sb)
                nc.sync.dma_start(p_d[j], p_sb)

    nc.finalize()
    return nc


def _get_program():
    if "nc" not in _CACHE:
        _CACHE["nc"] = _build_program()
    return _CACHE["nc"]


def kernel(r, feat, W1, b1, W2, b2, _run_kwargs=None, _return_results=False):
    from concourse.bass_utils import run_bass_kernel_spmd

    fdt = _np_feat_dtype()
    r = np.asarray(r, dtype=np.float32)
    feat = np.asarray(feat, dtype=np.float32)
    W1 = np.asarray(W1, dtype=np.float32)
    b1 = np.asarray(b1, dtype=np.float32).reshape(HID)
    W2 = np.asarray(W2, dtype=np.float32).reshape(HID)
    b2 = np.asarray(b2, dtype=np.float32).reshape(1)

    # host-side downcast + transpose: [T,B,F] -> [core, j, f, t_loc, b]
    featT = np.ascontiguousarray(
        feat.astype(fdt).reshape(NCORES, TLOC, NH, 128, FEAT)
            .transpose(0, 2, 4, 1, 3))
    # r: [T,B,1] -> [core, j, b, t_loc]
    rT = np.ascontiguousarray(
        r[:, :, 0].reshape(NCORES, TLOC, NH, 128).transpose(0, 2, 3, 1))
    w1c = np.ascontiguousarray(W1.astype(fdt))

    nc = _get_program()
    in_maps = []
    for c in range(NCORES):
        in_maps.append({
            "featT": featT[c], "rT": rT[c],
            "w1": w1c, "b1": b1, "w2": W2, "b2": b2,
        })

    kw = _run_kwargs or {}
    res = run_bass_kernel_spmd(nc, in_maps, core_ids=list(range(NCORES)), **kw)

    # host stitch: y = z + P*carry per slab, carry chain across slabs
    y = np.empty((T, B), dtype=np.float32)
    carry = r[0, :, 0].astype(np.float32)
    for c in range(NCORES):
        zc = res.results[c]["z"].transpose(2, 0, 1).reshape(TLOC, B)
        pc = res.results[c]["p"].transpose(2, 0, 1).reshape(TLOC, B)
        y_slab = zc + pc * carry[None, :]
        carry = y_slab[-1]
        y[c * TLOC:(c + 1) * TLOC] = y_slab
    out = y[:, :, None]
    if _return_results:
        return out, res
    return out


# revision 4
# speedup vs baseline: 1.9611x; 1.2418x over previous
"""EMA head kernel for Trainium2 (Bass/Tile), 8 NeuronCores.

Problem: alpha = clip(sigmoid(MLP(feat)), 0.01, 0.99) per (t, b);
         y[0] = r[0]; y[t] = (1-alpha[t])*y[t-1] + alpha[t]*r[t].

Sharding: time dim T=4096 split into 8 slabs of 512 (all B=256 per core).
Each core computes, for its slab, the local affine-scan pieces
    z[t] = A[t]*z[t-1] + Bv[t]   (z[-1] = 0),   A = 1-alpha, Bv = alpha*r
    P[t] = A[t]*P[t-1]           (P[-1] = 1)
and the host stitches slabs with   y = z + P * carry,  carry' = y[-1].
carry_0 = r[0] reproduces y[0] = r[0] exactly: a*r + (1-a)*r = r.

v2: feat is pre-transposed AND down-cast on the host to
    featT [2 (b-half j), 128 (f), TLOC (t), 128 (b)]  (fp16 or fp8).
Device DMA is then fully contiguous (16 KB per partition per chunk) and
the PE transpose pass + PSUM->SBUF copies of v1 disappear entirely:
matmul lhsT = featT tile [f, b] directly, rhs = W1 [f, 16]
-> h [128 b, 16] per t in PSUM, then +b1/relu/*W2/reduce -> alpha_pre,
sigmoid+clip, tensor_tensor_scan along t for z and P (as v1).
r is likewise pre-transposed on the host to rT [2, 128 b, TLOC] fp32.
"""

import numpy as np

T, B, FEAT, HID = 4096, 256, 128, 16
NCORES = 8
TLOC = T // NCORES  # 512
NH = 2              # batch halves of 128
CH = 64             # t-steps per feat DMA chunk (2 MB @ fp16)
SB = 32             # t-steps per PSUM h-bank (one 2KB bank)

FEAT_FP8 = True     # False: fp16 feat/W1.  True: fp8 (e4m3) feat/W1.

_CACHE = {}


def _np_feat_dtype():
    if FEAT_FP8:
        import ml_dtypes
        return ml_dtypes.float8_e4m3
    return np.float16


def _build_program():
    import concourse.bacc as bacc
    import concourse.bass as bass
    import concourse.tile as tile
    from concourse import mybir

    fp32 = mybir.dt.float32
    fdt = mybir.dt.float8e4 if FEAT_FP8 else mybir.dt.float16
    AF = mybir.ActivationFunctionType
    OP = mybir.AluOpType

    nc = bacc.Bacc("TRN2", target_bir_lowering=False, debug=False,
                   num_devices=NCORES)

    featT_d = nc.dram_tensor("featT", [NH, FEAT, TLOC, 128], fdt,
                             kind="ExternalInput")
    rT_d = nc.dram_tensor("rT", [NH, 128, TLOC], fp32, kind="ExternalInput")
    w1_d = nc.dram_tensor("w1", [FEAT, HID], fdt, kind="ExternalInput")
    b1_d = nc.dram_tensor("b1", [HID], fp32, kind="ExternalInput")
    w2_d = nc.dram_tensor("w2", [HID], fp32, kind="ExternalInput")
    b2_d = nc.dram_tensor("b2", [1], fp32, kind="ExternalInput")
    z_d = nc.dram_tensor("z", [NH, 128, TLOC], fp32, kind="ExternalOutput")
    p_d = nc.dram_tensor("p", [NH, 128, TLOC], fp32, kind="ExternalOutput")

    with tile.TileContext(nc) as tc:
        with (
            tc.tile_pool(name="singles", bufs=1) as singles,
            tc.tile_pool(name="featin", bufs=3) as featin,
            tc.tile_pool(name="hps", bufs=4, space="PSUM") as hps,
            tc.tile_pool(name="hwork", bufs=3) as hwork,
        ):
            # ---------------- constants ----------------
            w1_sb = singles.tile([128, HID], fdt)
            nc.sync.dma_start(w1_sb, w1_d[:, :])
            b1rep = singles.tile([128, SB, HID], fp32)
            nc.gpsimd.dma_start(
                b1rep, bass.AP(b1_d, 0, [[0, 128], [0, SB], [1, HID]]))
            w2rep = singles.tile([128, SB, HID], fp32)
            nc.gpsimd.dma_start(
                w2rep, bass.AP(w2_d, 0, [[0, 128], [0, SB], [1, HID]]))
            b2col = singles.tile([128, 1], fp32)
            nc.gpsimd.dma_start(b2col, bass.AP(b2_d, 0, [[0, 128], [1, 1]]))
            ones_sb = singles.tile([128, TLOC], fp32)
            nc.vector.memset(ones_sb, 1.0)

            rT = [singles.tile([128, TLOC], fp32, tag=f"rT{h}", name=f"rT{h}")
                  for h in range(NH)]
            for h in range(NH):
                nc.sync.dma_start(rT[h], rT_d[h])

            apre = [singles.tile([128, TLOC], fp32, tag=f"apre{h}",
                                 name=f"apre{h}")
                    for h in range(NH)]

            # ---------------- main feat pipeline ----------------
            dma_parity = 0
            for j in range(NH):
                for t0 in range(0, TLOC, CH):
                    ft = featin.tile([128, CH, 128], fdt, tag="ft")
                    eng = nc.sync if dma_parity == 0 else nc.gpsimd
                    dma_parity ^= 1
                    eng.dma_start(ft, featT_d[j, :, t0:t0 + CH, :])
                    for s2 in range(CH // SB):
                        hbank = hps.tile([128, SB, HID], fp32, tag="hb")
                        for s in range(SB):
                            nc.tensor.matmul(hbank[:, s, :],
                                             ft[:, s2 * SB + s, :], w1_sb)
                        blk = t0 + s2 * SB
                        hb = hwork.tile([128, SB, HID], fp32, tag="hadd")
                        nc.vector.tensor_add(hb, hbank, b1rep)
                        hrelu = hwork.tile([128, SB, HID], fp32, tag="hrelu")
                        nc.scalar.activation(hrelu, hb, AF.Relu)
                        hw = hwork.tile([128, SB, HID], fp32, tag="hw")
                        nc.vector.tensor_mul(hw, hrelu, w2rep)
                        nc.vector.tensor_reduce(
                            apre[j][:, blk:blk + SB], hw,
                            axis=mybir.AxisListType.X, op=OP.add)

                # ---------------- alpha -> scans -> out (half j) --------
                alpha = singles.tile([128, TLOC], fp32, tag=f"alpha{j}")
                nc.scalar.activation(alpha, apre[j], AF.Sigmoid, bias=b2col)
                nc.vector.tensor_scalar(alpha, alpha, 0.01, 0.99,
                                        op0=OP.max, op1=OP.min)
                A_sb = singles.tile([128, TLOC], fp32, tag=f"A{j}")
                nc.vector.tensor_scalar(A_sb, alpha, -1.0, 1.0,
                                        op0=OP.mult, op1=OP.add)
                Bv = singles.tile([128, TLOC], fp32, tag=f"Bv{j}")
                nc.vector.tensor_mul(Bv, alpha, rT[j])
                z_sb = singles.tile([128, TLOC], fp32, tag=f"z{j}")
                nc.vector.tensor_tensor_scan(z_sb, A_sb, Bv, 0.0,
                                             op0=OP.mult, op1=OP.add)
                p_sb = singles.tile([128, TLOC], fp32, tag=f"p{j}")
                nc.vector.tensor_tensor_scan(p_sb, A_sb, ones_sb, 1.0,
                                             op0=OP.mult, op1=OP.mult)
                nc.sync.dma_start(z_d[j], z_sb)
                nc.sync.dma_start(p_d[j], p_sb)

    nc.finalize()
    return nc


def _get_program():
    if "nc" not in _CACHE:
        _CACHE["nc"] = _build_program()
    return _CACHE["nc"]


def kernel(r, feat, W1, b1, W2, b2, _run_kwargs=None, _return_results=False):
    from concourse.bass_utils import run_bass_kernel_spmd

    fdt = _np_feat_dtype()
    r = np.asarray(r, dtype=np.float32)
    feat = np.asarray(feat, dtype=np.float32)
    W1 = np.asarray(W1, dtype=np.float32)
    b1 = np.asarray(b1, dtype=np.float32).reshape(HID)
    W2 = np.asarray(W2, dtype=np.float32).reshape(HID)
    b2 = np.asarray(b2, dtype=np.float32).reshape(1)

    # host-side downcast + transpose: [T,B,F] -> [core, j, f, t_loc, b]
    featT = np.ascontiguousarray(
        feat.astype(fdt).reshape(NCORES, TLOC, NH, 128, FEAT)
            .transpose(0, 2, 4, 1, 3))
    # r: [T,B,1] -> [core, j, b, t_loc]
    rT = np.ascontiguousarray(
        r[:, :, 0].reshape(NCORES, TLOC, NH, 128).transpose(0, 2, 3, 1))
    w1c = np.ascontiguousarray(W1.astype(fdt))

    nc = _get_program()
    in_maps = []
    for c in range(NCORES):
        in_maps.append({
            "featT": featT[c], "rT": rT[c],
            "w1": w1c, "b1": b1, "w2": W2, "b2": b2,
        })

    kw = _run_kwargs or {}
    res = run_bass_kernel_spmd(nc, in_maps, core_ids=list(range(NCORES)), **kw)

    # host stitch: y = z + P*carry per slab, carry chain across slabs
    y = np.empty((T, B), dtype=np.float32)
    carry = r[0, :, 0].astype(np.float32)
    for c in range(NCORES):
        zc = res.results[c]["z"].transpose(2, 0, 1).reshape(TLOC, B)
        pc = res.results[c]["p"].transpose(2, 0, 1).reshape(TLOC, B)
        y_slab = zc + pc * carry[None, :]
        carry = y_slab[-1]
        y[c * TLOC:(c + 1) * TLOC] = y_slab
    out = y[:, :, None]
    if _return_results:
        return out, res
    return out


# revision 5
# speedup vs baseline: 2.5892x; 1.3203x over previous
"""EMA head kernel for Trainium2 (Bass/Tile), 8 NeuronCores.

Problem: alpha = clip(sigmoid(MLP(feat)), 0.01, 0.99) per (t, b);
         y[0] = r[0]; y[t] = (1-alpha[t])*y[t-1] + alpha[t]*r[t].

Sharding: time dim T=4096 split into 8 slabs of 512 (all B=256 per core).
Each core computes, for its slab, the local affine-scan pieces
    z[t] = A[t]*z[t-1] + Bv[t]   (z[-1] = 0),   A = 1-alpha, Bv = alpha*r
    P[t] = A[t]*P[t-1]           (P[-1] = 1)
and the host stitches slabs with   y = z + P * carry,  carry' = y[-1].

v4: feat is pre-transposed + down-cast to fp8 (e4m3) on the host:
    featT [2 (b-half j), 128 (f), TLOC (t), 128 (b)].
Device DMA is fully contiguous (32 KB per partition per 4 MB chunk,
tapered at the end of each half to shrink the pipeline tail).
b1 is pre-added into PSUM via a K=1 ones-matmul per bank so ACT's relu
reads the matmul output directly (no DVE bias add).  h intermediates in
fp16 for 2x DVE.  alpha/scan tail is processed in 128-t blocks with
chained tensor_tensor_scan so only the last block remains after the
final feat chunk lands.  z/P are written out as fp16 per block.
"""

import numpy as np

T, B, FEAT, HID = 4096, 256, 128, 16
NCORES = 8
TLOC = T // NCORES  # 512
NH = 2              # batch halves of 128
SB = 32             # t-steps per PSUM h-bank (one 2KB bank)
TB = 128            # t-steps per alpha/scan block
CHUNKS = [256, 128, 64, 32, 32]   # t-steps per feat DMA chunk (per half)

FEAT_FP8 = True     # False: fp16 feat/W1.  True: fp8 (e4m3) feat/W1.

_CACHE = {}


def _np_feat_dtype():
    if FEAT_FP8:
        import ml_dtypes
        return ml_dtypes.float8_e4m3
    return np.float16


def _build_program():
    import concourse.bacc as bacc
    import concourse.bass as bass
    import concourse.tile as tile
    from concourse import mybir

    fp32 = mybir.dt.float32
    fp16 = mybir.dt.float16
    fdt = mybir.dt.float8e4 if FEAT_FP8 else mybir.dt.float16
    AF = mybir.ActivationFunctionType
    OP = mybir.AluOpType

    nc = bacc.Bacc("TRN2", target_bir_lowering=False, debug=False,
                   num_devices=NCORES)

    featT_d = nc.dram_tensor("featT", [NH, FEAT, TLOC, 128], fdt,
                             kind="ExternalInput")
    rT_d = nc.dram_tensor("rT", [NH, 128, TLOC], fp32, kind="ExternalInput")
    w1_d = nc.dram_tensor("w1", [FEAT, HID], fdt, kind="ExternalInput")
    b1_d = nc.dram_tensor("b1", [HID], fp32, kind="ExternalInput")
    w2_d = nc.dram_tensor("w2", [HID], fp32, kind="ExternalInput")
    b2_d = nc.dram_tensor("b2", [1], fp32, kind="ExternalInput")
    z_d = nc.dram_tensor("z", [NH, 128, TLOC], fp16, kind="ExternalOutput")
    p_d = nc.dram_tensor("p", [NH, 128, TLOC], fp16, kind="ExternalOutput")

    with tile.TileContext(nc) as tc:
        with (
            tc.tile_pool(name="singles", bufs=1) as singles,
            tc.tile_pool(name="featin", bufs=3) as featin,
            tc.tile_pool(name="hps", bufs=4, space="PSUM") as hps,
            tc.tile_pool(name="hwork", bufs=3) as hwork,
            tc.tile_pool(name="apool", bufs=2) as apool,
        ):
            # ---- constants: all on the scalar (ACT) HWDGE queue so they
            # cannot queue behind the multi-MB feat chunks on sync/gpsimd.
            w1_sb = singles.tile([128, HID], fdt)
            nc.scalar.dma_start(w1_sb, w1_d[:, :])
            # b1 replicated along free dim on ONE partition (for the K=1
            # bias preload matmul), fp16 for PE.
            b1row = singles.tile([1, SB, HID], fp16)
            nc.gpsimd.dma_start(
                b1row, bass.AP(b1_d, 0, [[0, 1], [0, SB], [1, HID]]))
            ones1 = singles.tile([1, 128], fp16)
            nc.vector.memset(ones1, 1.0)
            w2rep = singles.tile([128, SB, HID], fp16)
            nc.gpsimd.dma_start(
                w2rep, bass.AP(w2_d, 0, [[0, 128], [0, SB], [1, HID]]))
            b2col = singles.tile([128, 1], fp32)
            nc.gpsimd.dma_start(b2col, bass.AP(b2_d, 0, [[0, 128], [1, 1]]))
            ones_tb = singles.tile([128, TB], fp32)
            nc.vector.memset(ones_tb, 1.0)

            rT = [singles.tile([128, TLOC], fp32, tag=f"rT{h}", name=f"rT{h}")
                  for h in range(NH)]
            for h in range(NH):
                nc.scalar.dma_start(rT[h], rT_d[h])

            apre = [singles.tile([128, TLOC], fp32, tag=f"apre{h}",
                                 name=f"apre{h}")
                    for h in range(NH)]
            z_sb = [singles.tile([128, TLOC], fp16, tag=f"z{h}", name=f"z{h}")
                    for h in range(NH)]
            p_sb = [singles.tile([128, TLOC], fp16, tag=f"p{h}", name=f"p{h}")
                    for h in range(NH)]

            # ---------------- main feat pipeline ----------------
            dma_parity = 0

            def do_block(j, blk):
                """alpha -> A,Bv -> chained scans for t in [blk, blk+TB)."""
                al = apool.tile([128, TB], fp32, tag="al")
                nc.scalar.activation(al, apre[j][:, blk:blk + TB],
                                     AF.Sigmoid, bias=b2col)
                nc.vector.tensor_scalar(al, al, 0.01, 0.99,
                                        op0=OP.max, op1=OP.min)
                A_b = apool.tile([128, TB], fp32, tag="A")
                nc.vector.tensor_scalar(A_b, al, -1.0, 1.0,
                                        op0=OP.mult, op1=OP.add)
                Bv = apool.tile([128, TB], fp32, tag="Bv")
                nc.vector.tensor_mul(Bv, al, rT[j][:, blk:blk + TB])
                z0 = 0.0 if blk == 0 else z_sb[j][:, blk - 1:blk]
                nc.vector.tensor_tensor_scan(
                    z_sb[j][:, blk:blk + TB], A_b, Bv, z0,
                    op0=OP.mult, op1=OP.add)
                p0 = 1.0 if blk == 0 else p_sb[j][:, blk - 1:blk]
                nc.vector.tensor_tensor_scan(
                    p_sb[j][:, blk:blk + TB], A_b, ones_tb, p0,
                    op0=OP.mult, op1=OP.mult)
                nc.scalar.dma_start(z_d[j][:, blk:blk + TB],
                                    z_sb[j][:, blk:blk + TB])
                nc.scalar.dma_start(p_d[j][:, blk:blk + TB],
                                    p_sb[j][:, blk:blk + TB])

            for j in range(NH):
                t0 = 0
                for ch in CHUNKS:
                    ft = featin.tile([128, ch, 128], fdt, tag="ft")
                    eng = nc.sync if dma_parity == 0 else nc.gpsimd
                    dma_parity ^= 1
                    eng.dma_start(ft, featT_d[j, :, t0:t0 + ch, :])
                    for s2 in range(ch // SB):
                        blk = t0 + s2 * SB
                        hbank = hps.tile([128, SB, HID], fp32, tag="hb")
                        # preload b1 into the bank, then accumulate the
                        # 32 per-t matmuls on top of it.
                        nc.tensor.matmul(hbank, ones1, b1row,
                                         start=True, stop=False)
                        for s in range(SB):
                            nc.tensor.matmul(hbank[:, s, :],
                                             ft[:, s2 * SB + s, :], w1_sb,
                                             start=False, stop=(s == SB - 1))
                        hrelu = hwork.tile([128, SB, HID], fp16, tag="hrelu")
                        nc.scalar.activation(hrelu, hbank, AF.Relu)
                        hw = hwork.tile([128, SB, HID], fp16, tag="hw")
                        nc.vector.tensor_mul(hw, hrelu, w2rep)
                        nc.vector.tensor_reduce(
                            apre[j][:, blk:blk + SB], hw,
                            axis=mybir.AxisListType.X, op=OP.add)
                        if (blk + SB) % TB == 0:
                            do_block(j, blk + SB - TB)
                    t0 += ch

    nc.finalize()
    return nc


def _get_program():
    if "nc" not in _CACHE:
        _CACHE["nc"] = _build_program()
    return _CACHE["nc"]


def kernel(r, feat, W1, b1, W2, b2, _run_kwargs=None, _return_results=False):
    from concourse.bass_utils import run_bass_kernel_spmd

    fdt = _np_feat_dtype()
    r = np.asarray(r, dtype=np.float32)
    feat = np.asarray(feat, dtype=np.float32)
    W1 = np.asarray(W1, dtype=np.float32)
    b1 = np.asarray(b1, dtype=np.float32).reshape(HID)
    W2 = np.asarray(W2, dtype=np.float32).reshape(HID)
    b2 = np.asarray(b2, dtype=np.float32).reshape(1)

    # host-side downcast + transpose: [T,B,F] -> [core, j, f, t_loc, b]
    featT = np.ascontiguousarray(
        feat.astype(fdt).reshape(NCORES, TLOC, NH, 128, FEAT)
            .transpose(0, 2, 4, 1, 3))
    # r: [T,B,1] -> [core, j, b, t_loc]
    rT = np.ascontiguousarray(
        r[:, :, 0].reshape(NCORES, TLOC, NH, 128).transpose(0, 2, 3, 1))
    w1c = np.ascontiguousarray(W1.astype(fdt))

    nc = _get_program()
    in_maps = []
    for c in range(NCORES):
        in_maps.append({
            "featT": featT[c], "rT": rT[c],
            "w1": w1c, "b1": b1, "w2": W2, "b2": b2,
        })

    kw = _run_kwargs or {}
    res = run_bass_kernel_spmd(nc, in_maps, core_ids=list(range(NCORES)), **kw)

    # host stitch: y = z + P*carry per slab, carry chain across slabs
    y = np.empty((T, B), dtype=np.float32)
    carry = r[0, :, 0].astype(np.float32)
    for c in range(NCORES):
        zc = res.results[c]["z"].astype(np.float32).transpose(2, 0, 1)
        pc = res.results[c]["p"].astype(np.float32).transpose(2, 0, 1)
        zc = zc.reshape(TLOC, B)
        pc = pc.reshape(TLOC, B)
        y_slab = zc + pc * carry[None, :]
        carry = y_slab[-1]
        y[c * TLOC:(c + 1) * TLOC] = y_slab
    out = y[:, :, None]
    if _return_results:
        return out, res
    return out


# revision 7
# speedup vs baseline: 2.6684x; 1.0306x over previous
"""EMA head kernel for Trainium2 (Bass/Tile), 8 NeuronCores.

Problem: alpha = clip(sigmoid(MLP(feat)), 0.01, 0.99) per (t, b);
         y[0] = r[0]; y[t] = (1-alpha[t])*y[t-1] + alpha[t]*r[t].

Sharding: time dim T=4096 split into 8 slabs of 512 (all B=256 per core).
Each core computes, for its slab, the local affine-scan pieces
    z[t] = A[t]*z[t-1] + Bv[t]   (z[-1] = 0),   A = 1-alpha, Bv = alpha*r
    P[t] = A[t]*P[t-1]           (P[-1] = 1)
and the host stitches slabs with   y = z + P * carry,  carry' = y[-1].

v4: feat is pre-transposed + down-cast to fp8 (e4m3) on the host:
    featT [2 (b-half j), 128 (f), TLOC (t), 128 (b)].
Device DMA is fully contiguous (32 KB per partition per 4 MB chunk,
tapered at the end of each half to shrink the pipeline tail).
b1 is pre-added into PSUM via a K=1 ones-matmul per bank so ACT's relu
reads the matmul output directly (no DVE bias add).  h intermediates in
fp16 for 2x DVE.  alpha/scan tail is processed in 128-t blocks with
chained tensor_tensor_scan so only the last block remains after the
final feat chunk lands.  z/P are written out as fp16 per block.
"""

import numpy as np

T, B, FEAT, HID = 4096, 256, 128, 16
NCORES = 8
TLOC = T // NCORES  # 512
NH = 2              # batch halves of 128
SB = 32             # t-steps per PSUM h-bank (one 2KB bank)
TB = 128            # t-steps per alpha/scan block
CHUNKS = [256, 128, 64, 32, 32]   # t-steps per feat DMA chunk (per half)

FEAT_FP8 = True     # False: fp16 feat/W1.  True: fp8 (e4m3) feat/W1.

_CACHE = {}


def _np_feat_dtype():
    if FEAT_FP8:
        import ml_dtypes
        return ml_dtypes.float8_e4m3
    return np.float16


def _build_program():
    import concourse.bacc as bacc
    import concourse.bass as bass
    import concourse.tile as tile
    from concourse import mybir

    fp32 = mybir.dt.float32
    fp16 = mybir.dt.float16
    fdt = mybir.dt.float8e4 if FEAT_FP8 else mybir.dt.float16
    AF = mybir.ActivationFunctionType
    OP = mybir.AluOpType

    nc = bacc.Bacc("TRN2", target_bir_lowering=False, debug=False,
                   num_devices=NCORES)

    featT_d = nc.dram_tensor("featT", [NH, FEAT, TLOC, 128], fdt,
                             kind="ExternalInput")
    rT_d = nc.dram_tensor("rT", [NH, 128, TLOC], fp32, kind="ExternalInput")
    w1_d = nc.dram_tensor("w1", [FEAT, HID], fdt, kind="ExternalInput")
    b1_d = nc.dram_tensor("b1", [HID], fp32, kind="ExternalInput")
    w2_d = nc.dram_tensor("w2", [HID], fp32, kind="ExternalInput")
    b2_d = nc.dram_tensor("b2", [1], fp32, kind="ExternalInput")
    z_d = nc.dram_tensor("z", [NH, 128, TLOC], fp16, kind="ExternalOutput")
    p_d = nc.dram_tensor("p", [NH, 128, TLOC], fp16, kind="ExternalOutput")

    with tile.TileContext(nc) as tc:
        with (
            tc.tile_pool(name="singles", bufs=1) as singles,
            tc.tile_pool(name="featin", bufs=3) as featin,
            tc.tile_pool(name="hps", bufs=4, space="PSUM") as hps,
            tc.tile_pool(name="hwork", bufs=3) as hwork,
            tc.tile_pool(name="apool", bufs=2) as apool,
        ):
            # ---- constants: all on the scalar (ACT) HWDGE queue so they
            # cannot queue behind the multi-MB feat chunks on sync/gpsimd.
            w1_sb = singles.tile([128, HID], fdt)
            nc.scalar.dma_start(w1_sb, w1_d[:, :])
            # b1 replicated along free dim on ONE partition (for the K=1
            # bias preload matmul), fp16 for PE.
            b1row = singles.tile([1, SB, HID], fp16)
            nc.gpsimd.dma_start(
                b1row, bass.AP(b1_d, 0, [[0, 1], [0, SB], [1, HID]]))
            ones1 = singles.tile([1, 128], fp16)
            nc.vector.memset(ones1, 1.0)
            w2rep = singles.tile([128, SB, HID], fp16)
            nc.gpsimd.dma_start(
                w2rep, bass.AP(w2_d, 0, [[0, 128], [0, SB], [1, HID]]))
            b2col = singles.tile([128, 1], fp32)
            nc.gpsimd.dma_start(b2col, bass.AP(b2_d, 0, [[0, 128], [1, 1]]))
            ones_tb = singles.tile([128, TB], fp32)
            nc.vector.memset(ones_tb, 1.0)

            rT = [singles.tile([128, TLOC], fp32, tag=f"rT{h}", name=f"rT{h}")
                  for h in range(NH)]
            for h in range(NH):
                nc.scalar.dma_start(rT[h], rT_d[h])

            apre = [singles.tile([128, TLOC], fp32, tag=f"apre{h}",
                                 name=f"apre{h}")
                    for h in range(NH)]
            z_sb = [singles.tile([128, TLOC], fp16, tag=f"z{h}", name=f"z{h}")
                    for h in range(NH)]
            p_sb = [singles.tile([128, TLOC], fp16, tag=f"p{h}", name=f"p{h}")
                    for h in range(NH)]

            # every feat chunk gets its own dedicated buffer (16 MB total)
            # so ALL feat dma_starts issue immediately with no reuse waits.
            ft_tiles = {}
            dma_parity = 0
            for j in range(NH):
                t0 = 0
                for ci, ch in enumerate(CHUNKS):
                    ft = singles.tile([128, ch, 128], fdt, tag=f"ft{j}_{ci}",
                                      name=f"ft{j}_{ci}")
                    eng = nc.sync if dma_parity == 0 else nc.gpsimd
                    dma_parity ^= 1
                    eng.dma_start(ft, featT_d[j, :, t0:t0 + ch, :])
                    ft_tiles[(j, ci)] = ft
                    t0 += ch

            # ---------------- main feat pipeline ----------------
            dma_parity = 0

            def do_block(j, blk):
                """alpha -> A,Bv -> chained scans for t in [blk, blk+TB)."""
                al = apool.tile([128, TB], fp32, tag="al")
                nc.scalar.activation(al, apre[j][:, blk:blk + TB],
                                     AF.Sigmoid, bias=b2col)
                nc.vector.tensor_scalar(al, al, 0.01, 0.99,
                                        op0=OP.max, op1=OP.min)
                A_b = apool.tile([128, TB], fp32, tag="A")
                nc.vector.tensor_scalar(A_b, al, -1.0, 1.0,
                                        op0=OP.mult, op1=OP.add)
                Bv = apool.tile([128, TB], fp32, tag="Bv")
                nc.vector.tensor_mul(Bv, al, rT[j][:, blk:blk + TB])
                z0 = 0.0 if blk == 0 else z_sb[j][:, blk - 1:blk]
                nc.vector.tensor_tensor_scan(
                    z_sb[j][:, blk:blk + TB], A_b, Bv, z0,
                    op0=OP.mult, op1=OP.add)
                p0 = 1.0 if blk == 0 else p_sb[j][:, blk - 1:blk]
                nc.vector.tensor_tensor_scan(
                    p_sb[j][:, blk:blk + TB], A_b, ones_tb, p0,
                    op0=OP.mult, op1=OP.mult)
                if blk + TB == TLOC:
                    # whole half done: write z/p on the (now idle) big
                    # queues, in parallel.
                    nc.sync.dma_start(z_d[j], z_sb[j])
                    nc.gpsimd.dma_start(p_d[j], p_sb[j])

            for j in range(NH):
                t0 = 0
                for ci, ch in enumerate(CHUNKS):
                    ft = ft_tiles[(j, ci)]
                    for s2 in range(ch // SB):
                        blk = t0 + s2 * SB
                        hbank = hps.tile([128, SB, HID], fp32, tag="hb")
                        # preload b1 into the bank, then accumulate the
                        # 32 per-t matmuls on top of it.
                        nc.tensor.matmul(hbank, ones1, b1row,
                                         start=True, stop=False)
                        for s in range(SB):
                            nc.tensor.matmul(hbank[:, s, :],
                                             ft[:, s2 * SB + s, :], w1_sb,
                                             start=False, stop=(s == SB - 1))
                        hrelu = hwork.tile([128, SB, HID], fp16, tag="hrelu")
                        nc.scalar.activation(hrelu, hbank, AF.Relu)
                        hw = hwork.tile([128, SB, HID], fp16, tag="hw")
                        nc.vector.tensor_mul(hw, hrelu, w2rep)
                        nc.vector.tensor_reduce(
                            apre[j][:, blk:blk + SB], hw,
                            axis=mybir.AxisListType.X, op=OP.add)
                        if (blk + SB) % TB == 0:
                            do_block(j, blk + SB - TB)
                    t0 += ch

    nc.finalize()
    return nc


def _get_program():
    if "nc" not in _CACHE:
        _CACHE["nc"] = _build_program()
    return _CACHE["nc"]


def kernel(r, feat, W1, b1, W2, b2, _run_kwargs=None, _return_results=False):
    from concourse.bass_utils import run_bass_kernel_spmd

    fdt = _np_feat_dtype()
    r = np.asarray(r, dtype=np.float32)
    feat = np.asarray(feat, dtype=np.float32)
    W1 = np.asarray(W1, dtype=np.float32)
    b1 = np.asarray(b1, dtype=np.float32).reshape(HID)
    W2 = np.asarray(W2, dtype=np.float32).reshape(HID)
    b2 = np.asarray(b2, dtype=np.float32).reshape(1)

    # host-side downcast + transpose: [T,B,F] -> [core, j, f, t_loc, b]
    featT = np.ascontiguousarray(
        feat.astype(fdt).reshape(NCORES, TLOC, NH, 128, FEAT)
            .transpose(0, 2, 4, 1, 3))
    # r: [T,B,1] -> [core, j, b, t_loc]
    rT = np.ascontiguousarray(
        r[:, :, 0].reshape(NCORES, TLOC, NH, 128).transpose(0, 2, 3, 1))
    w1c = np.ascontiguousarray(W1.astype(fdt))

    nc = _get_program()
    in_maps = []
    for c in range(NCORES):
        in_maps.append({
            "featT": featT[c], "rT": rT[c],
            "w1": w1c, "b1": b1, "w2": W2, "b2": b2,
        })

    kw = _run_kwargs or {}
    res = run_bass_kernel_spmd(nc, in_maps, core_ids=list(range(NCORES)), **kw)

    # host stitch: y = z + P*carry per slab, carry chain across slabs
    y = np.empty((T, B), dtype=np.float32)
    carry = r[0, :, 0].astype(np.float32)
    for c in range(NCORES):
        zc = res.results[c]["z"].astype(np.float32).transpose(2, 0, 1)
        pc = res.results[c]["p"].astype(np.float32).transpose(2, 0, 1)
        zc = zc.reshape(TLOC, B)
        pc = pc.reshape(TLOC, B)
        y_slab = zc + pc * carry[None, :]
        carry = y_slab[-1]
        y[c * TLOC:(c + 1) * TLOC] = y_slab
    out = y[:, :, None]
    if _return_results:
        return out, res
    return out
